# revision 40
# baseline (speedup 1.0000x reference)
"""DeepseekV2 MLA attention on 8 Trainium2 NeuronCores.

Sharding: token-split A-projections -> AllGather(kv latent, fired early) +
AllGather(q latent) -> head-split (4 heads/core) B-projections + causal
attention -> per-query-block AllGather(attn out) -> D-column-split output
projection. Layouts are d-major (feature dim on the SBUF partition axis).

Pipeline: the kv latent gather fires ~1/3 into phase_a so phase_b (k/v
projections) runs right after phase_a's matmuls; the q-latent gather is
hidden under phase_b; phase_q computes its rope chunks first so the
serialized vector rope chain overlaps the nope matmuls; q stays resident
in SBUF; the first o-projection input block is prefetched during
attention so phase_out starts without a DMA bubble.

Precision: bf16 matmul inputs with fp32 PSUM accumulation throughout;
rmsnorm statistics and softmax run in fp32/f32r.
"""
import math

import numpy as np
import ml_dtypes

import concourse.bass as bass
import concourse.mybir as mybir
from concourse.tile import TileContext
from concourse import bass_utils

# ---------------------------------------------------------------------------
# Walrus workaround: this container's walrus accepts at most ONE sync-wait
# per TPB instruction, but Tile attaches several (tail Drain, LDWEIGHTS...).
# Split: keep the last wait, move the rest onto preceding same-engine NOPs.
# ---------------------------------------------------------------------------
import concourse.tile as _tile_mod

_orig_sched = _tile_mod.TileContext.schedule_and_allocate
_nopctr = [0]


def _split_multiwait(nc):
    for fn in nc.m.functions:
        for blk in fn.blocks:
            insts = blk.instructions
            if not any(
                i.sync_info and i.sync_info.on_wait and len(i.sync_info.on_wait) > 1
                for i in insts
            ):
                continue
            out = []
            for ins in insts:
                si = ins.sync_info
                if si and si.on_wait and len(si.on_wait) > 1:
                    waits = list(si.on_wait)
                    for w in waits[:-1]:
                        _nopctr[0] += 1
                        nop = mybir.InstNoOp(name=f"I-mws-{_nopctr[0]}", ins=[], outs=[])
                        nop.engine = ins.engine
                        nop.sync_info = mybir.SyncInfo(on_wait=[w], on_update=[])
                        out.append(nop)
                    ins.sync_info = mybir.SyncInfo(
                        on_wait=[waits[-1]], on_update=list(si.on_update or [])
                    )
                out.append(ins)
            blk.instructions = out


def _patched_sched(self, *a, **k):
    res = _orig_sched(self, *a, **k)
    _split_multiwait(self.nc)
    return res


if getattr(_tile_mod.TileContext.schedule_and_allocate, "__name__", "") != "_patched_sched":
    _tile_mod.TileContext.schedule_and_allocate = _patched_sched


# ---------------------------------------------------------------------------
T, D, H = 2048, 5120, 32
NOPE, ROPE, QK = 128, 64, 192
KVR, QR, VH = 512, 1536, 128
EPS, THETA = 1e-6, 10000.0
NCORES = 8
HL = H // NCORES          # 4 heads per core
TC = T // NCORES          # 256 tokens per core
LAT = KVR + ROPE          # 576
DCOL = D // NCORES        # 640 output columns per core

F32 = mybir.dt.float32
F32R = mybir.dt.float32r
BF16 = mybir.dt.bfloat16
AF = mybir.ActivationFunctionType
MUL = mybir.AluOpType.mult
ADD = mybir.AluOpType.add
SUB = mybir.AluOpType.subtract

TRACE = [False]          # test.py sets TRACE[0]=True to profile
LAST_RESULT = [None]     # BassKernelResults stashed here for test.py

_cache = {}


def _phase_a(nc, tc, io, consts_t, agkv_in, agkv_out, agql_in, agql_out):
    """Token-split A projections with the h-chunk STATIONARY and the weight
    rows moving (512-wide matmuls; ~25% fewer PE cycles than the
    weight-stationary form, and no sum-of-squares / broadcast matmuls).
    Latents come out token-major; rmsnorm is a per-partition free-dim
    reduce; PE transposes restore d-major for the gathers.  The q latents
    ship UNNORMALIZED with the per-token 1/rms as an extra gathered row
    (applied consumer-side, folded into existing vector ops), so AG(q)
    fires right after the q matmuls with no normalization serialization."""
    with (
        tc.tile_pool(name="a_ht", bufs=1) as ht_pool,
        tc.tile_pool(name="a_cst", bufs=1) as a_cst,
        tc.tile_pool(name="a_w", bufs=3) as a_w,
        tc.tile_pool(name="a_st", bufs=2) as a_st,
        tc.tile_pool(name="a_tmp", bufs=2) as a_tmp,
        tc.tile_pool(name="a_ps", bufs=1, space="PSUM") as a_ps,
    ):
        ht_sb = ht_pool.tile([128, 40 * TC], BF16, name="ht_sb")
        htv = ht_sb[:].rearrange("p (k t) -> p k t", k=40)
        nc.sync.dma_start(htv, io["hT"][:].rearrange("(k p) t -> p k t", p=128))
        identb = a_cst.tile([128, 128], BF16, name="identb")
        nc.sync.dma_start(identb[:], io["identb"][:])
        identf = a_cst.tile([128, 128], F32, name="identf")
        nc.sync.dma_start(identf[:], io["identf"][:])
        biasr = a_cst.tile([128, LAT], F32, name="biasr")
        nc.sync.dma_start(biasr[:], io["biasrep"][:])
        cosat = a_cst.tile([128, 64], F32, name="cosat")
        sinat = a_cst.tile([128, 64], F32, name="sinat")
        for th in range(2):
            nc.sync.dma_start(cosat[:, th * 32:(th + 1) * 32],
                              io["cosAT"][th * 128:(th + 1) * 128, :])
            nc.sync.dma_start(sinat[:, th * 32:(th + 1) * 32],
                              io["sinAT"][th * 128:(th + 1) * 128, :])
        wav = io["wa"][:].rearrange("(k p) c -> p k c", p=128)

        def rms_scale(stin, nfeat, tag):
            """per-partition 1/sqrt(mean(stin^2)+eps) -> [128,1] f32r."""
            sq = a_st.tile([128, stin.shape[-1]], F32, name=f"sq_{tag}",
                           tag=f"sq_{tag}")
            nc.scalar.activation(sq[:], stin, AF.Square)
            ss = a_tmp.tile([128, 1], F32, name=f"ss_{tag}", tag=f"ss_{tag}")
            nc.vector.tensor_reduce(ss[:], sq[:], axis=mybir.AxisListType.X,
                                    op=ADD)
            ms = a_tmp.tile([128, 1], F32, name=f"ms_{tag}", tag=f"ms_{tag}")
            nc.vector.tensor_scalar(ms[:], ss[:], 1.0 / nfeat, EPS,
                                    op0=MUL, op1=ADD)
            sx = a_tmp.tile([128, 1], F32, name=f"sx_{tag}", tag=f"sx_{tag}")
            nc.scalar.activation(sx[:], ms[:], AF.Sqrt)
            rs = a_tmp.tile([128, 1], F32, name=f"rs_{tag}", tag=f"rs_{tag}")
            nc.vector.reciprocal(rs[:], sx[:])
            return rs

        def transpose_out(src_bf, cols, dst, dst_col0, th, tags):
            """PE-transpose [128, cols] bf16 (token-major) into d-major
            [cols, 128] and DMA to dst rows [dst_col0*...], token cols th."""
            nblk = (cols + 127) // 128
            for c in range(nblk):
                w = min(128, cols - c * 128)
                tp = a_ps.tile([128, 128], BF16, name=f"tp_{th}_{dst_col0}_{c}",
                               tag=tags[c % 2])
                nc.tensor.transpose(tp[:w, :], src_bf[:, c * 128:c * 128 + w],
                                    identb[:])
                tsb = a_tmp.tile([128, 128], BF16, name=f"tsb_{th}_{dst_col0}_{c}",
                                 tag="tsb")
                nc.vector.tensor_copy(tsb[:w, :], tp[:w, :])
                nc.sync.dma_start(
                    dst[dst_col0 + c * 128:dst_col0 + c * 128 + w,
                        th * 128:(th + 1) * 128], tsb[:w, :])

        # ---- pass 1: q columns (0..1535), both token halves ----
        psq = {}
        for th in range(2):
            for g in range(3):
                psq[th, g] = a_ps.tile([128, 512], F32, name=f"psq_{th}_{g}",
                                       tag=f"ps{th}{g}")
        for k in range(40):
            wt = a_w.tile([128, QR], BF16, name=f"aw_{k}", tag="aw")
            nc.sync.dma_start(wt[:], wav[:, k, 0:QR])
            for th in range(2):
                for g in range(3):
                    nc.tensor.matmul(psq[th, g][:],
                                     htv[:, k, th * 128:(th + 1) * 128],
                                     wt[:, g * 512:(g + 1) * 512],
                                     start=(k == 0), stop=(k == 39))
        qbfs = {}
        for th in range(2):
            qbf = a_st.tile([128, QR], BF16, name=f"qbf_{th}", tag="qbf")
            for g in range(3):
                nc.vector.tensor_copy(qbf[:, g * 512:(g + 1) * 512],
                                      psq[th, g][:])
            qbfs[th] = qbf
        srow = a_tmp.tile([1, TC], BF16, name="srow")
        for th in range(2):
            rs = rms_scale(qbfs[th][:], QR, f"q{th}")
            sps = a_ps.tile([1, 128], F32, name=f"sps_{th}", tag="pssc")
            nc.tensor.matmul(sps[:], rs[:], identf[:], start=True, stop=True)
            nc.vector.tensor_copy(srow[:, th * 128:(th + 1) * 128], sps[:])
        for th in range(2):
            transpose_out(qbfs[th][:], QR, agql_in, 0, th, ("ps00", "ps01"))
        nc.sync.dma_start(agql_in[QR:QR + 1, :], srow[:])
        with nc.named_scope("ag_ql"):
            nc.gpsimd.collective_compute(
                "AllGather", mybir.AluOpType.bypass,
                ins=[agql_in[:]], outs=[agql_out[:]],
                replica_groups=[list(range(NCORES))],
            )

        # ---- pass 2: kv columns (1536..2111) ----
        pskv = {}
        for th in range(2):
            pskv[th, 0] = a_ps.tile([128, 512], F32, name=f"pskv_{th}_0",
                                    tag=f"ps{th}0")
            pskv[th, 1] = a_ps.tile([128, 64], F32, name=f"pskv_{th}_1",
                                    tag=f"ps{th}1")
        for k in range(40):
            wt2 = a_w.tile([128, LAT], BF16, name=f"awkv_{k}", tag="awkv")
            nc.sync.dma_start(wt2[:], wav[:, k, QR:QR + LAT])
            for th in range(2):
                nc.tensor.matmul(pskv[th, 0][:],
                                 htv[:, k, th * 128:(th + 1) * 128],
                                 wt2[:, 0:512], start=(k == 0), stop=(k == 39))
                nc.tensor.matmul(pskv[th, 1][:],
                                 htv[:, k, th * 128:(th + 1) * 128],
                                 wt2[:, 512:576], start=(k == 0), stop=(k == 39))
        for th in range(2):
            stkv = a_st.tile([128, LAT], F32, name=f"stkv_{th}", tag="stkv")
            nc.vector.tensor_tensor(stkv[:, 0:512], pskv[th, 0][:],
                                    biasr[:, 0:512], op=ADD)
            nc.vector.tensor_tensor(stkv[:, 512:576], pskv[th, 1][:],
                                    biasr[:, 512:576], op=ADD)
            rskv = rms_scale(stkv[:, 0:512], KVR, f"kv{th}")
            kvbf = a_tmp.tile([128, LAT], BF16, name=f"kvbf_{th}", tag="kvbf")
            nc.vector.tensor_scalar(kvbf[:, 0:512], stkv[:, 0:512], rskv[:],
                                    None, op0=MUL)
            x1, x2 = stkv[:, 512:544], stkv[:, 544:576]
            ca = cosat[:, th * 32:(th + 1) * 32]
            sa = sinat[:, th * 32:(th + 1) * 32]
            ct1 = a_tmp.tile([128, 32], F32, name=f"ct1_{th}", tag="ct1")
            ct2 = a_tmp.tile([128, 32], F32, name=f"ct2_{th}", tag="ct2")
            nc.vector.tensor_tensor(ct1[:], x1, ca, op=MUL)
            nc.vector.tensor_tensor(ct2[:], x2, sa, op=MUL)
            nc.vector.tensor_tensor(kvbf[:, 512:544], ct1[:], ct2[:], op=SUB)
            nc.vector.tensor_tensor(ct1[:], x1, sa, op=MUL)
            nc.vector.tensor_tensor(ct2[:], x2, ca, op=MUL)
            nc.vector.tensor_tensor(kvbf[:, 544:576], ct1[:], ct2[:], op=ADD)
            transpose_out(kvbf[:, 0:512], 512, agkv_in, 0, th, ("ps02", "ps12"))
            transpose_out(kvbf[:, 512:576], 64, agkv_in, 512, th,
                          ("ps02", "ps12"))
        with nc.named_scope("ag_kv"):
            nc.gpsimd.collective_compute(
                "AllGather", mybir.AluOpType.bypass,
                ins=[agkv_in[:]], outs=[agkv_out[:]],
                replica_groups=[list(range(NCORES))],
            )


def _phase_b_loads(nc, io, agkvv, kpe_sb, b_kva, b_w):
    """DMA the gathered kv latents + B-projection weights; returns tiles.
    Issued on the SCALAR hw-dma queue so they don't head-of-line block
    the sync queue (they wait on AG(kv), which lands late but with
    slack — phase_b's matmuls run after phase_q's)."""
    wk_sb = b_w.tile([128, 4 * 512], BF16, name="wk_sb", tag="wkw")
    nc.scalar.dma_start(wk_sb[:].rearrange("p (k c) -> p k c", k=4),
                        io["wkvbk"][:].rearrange("(k p) c -> p k c", p=128))
    kva_sb = b_kva.tile([128, 4 * T], BF16, name="kva_sb")
    kvav = kva_sb[:].rearrange("p (k t) -> p k t", k=4)
    for k in range(4):
        nc.scalar.dma_start(
            kvav[:, k, :].rearrange("p (r t) -> p r t", r=NCORES),
            agkvv[k * 128:(k + 1) * 128])
    # k_pe duplicated on both partition halves so attention can feed
    # matmuls whose q slice lives at base partition 0 or 64
    for half in range(2):
        nc.scalar.dma_start(
            kpe_sb[64 * half:64 * half + 64, :].rearrange(
                "p (r t) -> p r t", r=NCORES),
            agkvv[512:576])
    wv_sb = b_w.tile([128, 4 * 512], BF16, name="wv_sb", tag="wvw")
    nc.scalar.dma_start(wv_sb[:].rearrange("p (k c) -> p k c", k=4),
                        io["wkvbv"][:].rearrange("(k p) c -> p k c", p=128))
    return kva_sb, wk_sb, wv_sb


def _phase_b(nc, tc, ktv, vv, kva_sb, wk_sb, wv_sb):
    """Head-split k_nope^T and v projections from the gathered kv latents."""
    kvav = kva_sb[:].rearrange("p (k t) -> p k t", k=4)
    wkv_ = wk_sb[:].rearrange("p (k c) -> p k c", k=4)
    wvv = wv_sb[:].rearrange("p (k c) -> p k c", k=4)
    with tc.tile_pool(name="b_ps", bufs=2, space="PSUM") as b_ps:
        for j in range(HL):
            for qb in range(4):
                ps = b_ps.tile([128, 512], F32, name=f"psk_{j}_{qb}", tag="psk")
                for k in range(4):
                    nc.tensor.matmul(ps[:], wkv_[:, k, j * 128:(j + 1) * 128],
                                     kvav[:, k, qb * 512:(qb + 1) * 512],
                                     start=(k == 0), stop=(k == 3))
                nc.vector.tensor_copy(ktv[:, j, qb * 512:(qb + 1) * 512], ps[:])
        for mt in range(16):
            ps = b_ps.tile([128, 512], F32, name=f"psv_{mt}", tag="psv")
            for k in range(4):
                nc.tensor.matmul(ps[:], kvav[:, k, mt * 128:(mt + 1) * 128],
                                 wvv[:, k, :], start=(k == 0), stop=(k == 3))
            nc.vector.tensor_copy(vv[:, mt, :], ps[:])


def _phase_q(nc, tc, io, consts_t, agqlv, qt_sb, prefetch_cb):
    """Head-split q^T projection into resident SBUF qt; rope (pe) chunks
    are computed FIRST so the serialized vector rope chain overlaps the
    nope matmuls that follow."""
    qtv = qt_sb[:].rearrange("p (c t) -> p c t", c=6)
    with (
        tc.tile_pool(name="c_qa", bufs=1) as c_qa,
        tc.tile_pool(name="c_tab", bufs=1) as c_tab,
        tc.tile_pool(name="c_tmp", bufs=1) as c_tmp,
        tc.tile_pool(name="c_ps", bufs=2, space="PSUM") as c_ps,
    ):
        # gathered q latents: 12 chunks split across the two HW DMA queues
        # (Sync + Activation) so the strided gathers land ~2x faster
        qa_sb = c_qa.tile([128, 12 * T], BF16, name="qa_sb")
        qav = qa_sb[:].rearrange("p (k t) -> p k t", k=12)
        for k in range(12):
            eng = nc.sync if k % 2 == 0 else nc.scalar
            eng.dma_start(
                qav[:, k, :].rearrange("p (r t) -> p r t", r=NCORES),
                agqlv[k * 128:(k + 1) * 128])
        prefetch_cb()
        sbc = c_tab.tile([128, T], F32R, name="sbc")
        cos2 = c_tab.tile([128, T], F32R, name="cos2")
        sin2 = c_tab.tile([128, T], F32R, name="sin2")
        ones_r = consts_t["ones_r"]
        with tc.tile_pool(name="c_cs", bufs=1) as c_cs:
            # per-token 1/rms scale row (row QR of every rank block) ->
            # broadcast to 128 partitions, folded into cos/sin tables and
            # the nope copy-multiplies below
            srow_sb = c_cs.tile([1, T], BF16, name="srow_sb")
            nc.sync.dma_start(
                srow_sb[:].rearrange("p (r t) -> p r t", r=NCORES),
                agqlv[QR:QR + 1])
            srow_f = c_cs.tile([1, T], F32R, name="srow_f")
            nc.vector.tensor_copy(srow_f[:], srow_sb[:])
            for qq in range(4):
                bps = c_ps.tile([128, 512], F32, name=f"sbps_{qq}",
                                tag=f"psq{qq}")
                nc.tensor.matmul(bps[:], ones_r[:1, :],
                                 srow_f[:, qq * 512:(qq + 1) * 512],
                                 start=True, stop=True)
                nc.vector.tensor_copy(sbc[:, qq * 512:(qq + 1) * 512], bps[:])
            cos_sb = c_cs.tile([128, T], F32R, name="cos_sb")
            sin_sb = c_cs.tile([128, T], F32R, name="sin_sb")
            nc.sync.dma_start(cos_sb[:], io["cosT"][:])
            nc.sync.dma_start(sin_sb[:], io["sinT"][:])
            nc.vector.tensor_tensor(cos2[:], cos_sb[:], sbc[:], op=MUL)
            nc.vector.tensor_tensor(sin2[:], sin_sb[:], sbc[:], op=MUL)
        ctx_w = tc.tile_pool(name="c_w", bufs=6)
        c_w = ctx_w.__enter__()
        morder = (4, 5, 0, 1, 2, 3)   # pe chunks first
        wts = {}
        for m in morder:
            wt = c_w.tile([128, 12 * 128], BF16, name=f"cw_{m}", tag="cw")
            nc.sync.dma_start(
                wt[:].rearrange("p (k c) -> p k c", k=12),
                io["wqb"][:].rearrange("(k p) c -> p k c", p=128)[
                    :, :, m * 128:(m + 1) * 128])
            wts[m] = wt
        pestage = c_tab.tile([128, 2 * T], F32R, name="pestage")

        def rope_batch(m, qb):
            st = qtv[:, m, qb * 512:(qb + 1) * 512]
            pe = pestage[:, (m - 4) * T + qb * 512:(m - 4) * T + (qb + 1) * 512]
            cs = cos2[:, qb * 512:(qb + 1) * 512]
            sn = sin2[:, qb * 512:(qb + 1) * 512]
            for half in range(2):
                r0 = 64 * half
                x1 = pe[r0:r0 + 32, :]
                x2 = pe[r0 + 32:r0 + 64, :]
                t1 = c_tmp.tile([32, 512], F32R,
                                name=f"ct1_{m}_{qb}_{half}", tag="ct1")
                t2 = c_tmp.tile([32, 512], F32R,
                                name=f"ct2_{m}_{qb}_{half}", tag="ct2")
                nc.vector.tensor_tensor(t1[:], x1, cs[r0:r0 + 32, :], op=MUL)
                nc.vector.tensor_tensor(t2[:], x2, sn[r0 + 32:r0 + 64, :],
                                        op=MUL)
                nc.vector.tensor_tensor(st[r0:r0 + 32, :], t1[:], t2[:],
                                        op=SUB)
                nc.vector.tensor_tensor(t1[:], x1, sn[r0:r0 + 32, :], op=MUL)
                nc.vector.tensor_tensor(t2[:], x2, cs[r0 + 32:r0 + 64, :],
                                        op=MUL)
                nc.vector.tensor_tensor(st[r0 + 32:r0 + 64, :], t1[:], t2[:],
                                        op=ADD)

        # rope batches are interleaved between the nope chunks' PSUM copies
        # on the vector FIFO: each batch (~16us) paces one nope chunk's
        # matmuls (~15us) without ever blocking a PSUM hand-off
        rope_work = [(m, qb) for qb in range(4) for m in (4, 5)]
        for m in morder:
            wtv = wts[m][:].rearrange("p (k c) -> p k c", k=12)
            pss = [c_ps.tile([128, 512], F32, name=f"psq_{m}_{qb}", tag=f"psq{qb}")
                   for qb in range(4)]
            for k in range(12):
                for qb in range(4):
                    nc.tensor.matmul(pss[qb][:], wtv[:, k, :],
                                     qav[:, k, qb * 512:(qb + 1) * 512],
                                     start=(k == 0), stop=(k == 11))
            for qb in range(4):
                if m < 4:
                    nc.vector.tensor_tensor(qtv[:, m, qb * 512:(qb + 1) * 512],
                                            pss[qb][:],
                                            sbc[:, qb * 512:(qb + 1) * 512],
                                            op=MUL)
                else:
                    pe = pestage[:, (m - 4) * T + qb * 512:
                                 (m - 4) * T + (qb + 1) * 512]
                    nc.vector.tensor_copy(pe, pss[qb][:])
            if m < 4 and rope_work:
                rope_batch(*rope_work.pop(0))
                rope_batch(*rope_work.pop(0))
        for mq in rope_work:
            rope_batch(*mq)
        ctx_w.__exit__(None, None, None)


def _phase_attn(nc, tc, qt_sb, ag2_ins, ag2_outs, ktv, vv, kpe_sb, consts_t,
                oa0):
    """Causal attention, two heads interleaved per pass; bf16 out -> ag2_in.
    q is read directly from resident SBUF (qt_sb)."""
    ones_c, ones_r, tri_sb = (consts_t["ones_cb"], consts_t["ones_r"],
                              consts_t["tri_b"])
    with (
        tc.tile_pool(name="t_p", bufs=8) as t_p,
        tc.tile_pool(name="t_o", bufs=2) as t_o,
        tc.tile_pool(name="t_ps", bufs=3, space="PSUM") as t_ps,
        tc.tile_pool(name="t_bc", bufs=1, space="PSUM") as t_bc,
        tc.tile_pool(name="t_acc", bufs=1, space="PSUM") as t_acc,
    ):
        for qb in range(4):
            for jp in range(HL // 2):
                js = (2 * jp, 2 * jp + 1)
                qf = {}
                dens, ots = {}, {}
                for s, j in enumerate(js):
                    qfn = qt_sb[:, j * T + qb * 512:j * T + (qb + 1) * 512]
                    pc = (4 + j // 2) * T + qb * 512
                    r0 = 64 * (j % 2)
                    qfp = qt_sb[r0:r0 + 64, pc:pc + 512]
                    qf[j] = (qfn, qfp)
                    dens[j] = t_acc.tile([1, 512], F32, name=f"den_{qb}_{j}",
                                         tag=f"den{s}")
                    ots[j] = t_acc.tile([128, 512], F32, name=f"ot_{qb}_{j}",
                                        tag=f"ot{s}")
                kmax = 4 * qb + 4
                for kk in range(kmax):
                    o = kk - 4 * qb
                    c0 = max(0, o) * 128
                    pts = {}
                    for s, j in enumerate(js):
                        qfn, qfp = qf[j]
                        sT = t_ps.tile([128, 512], F32,
                                       name=f"sT_{qb}_{j}_{kk}", tag="sT")
                        nc.tensor.matmul(sT[:, c0:512],
                                         ktv[:, j, kk * 128:(kk + 1) * 128],
                                         qfn[:, c0:512], start=True, stop=False)
                        r0 = 64 * (j % 2)
                        nc.tensor.matmul(sT[:, c0:512],
                                         kpe_sb[r0:r0 + 64,
                                                kk * 128:(kk + 1) * 128],
                                         qfp[:, c0:512], start=False, stop=True)
                        pT = t_p.tile([128, 512], BF16,
                                      name=f"pT_{qb}_{j}_{kk}", tag="pT")
                        nc.scalar.activation(pT[:, c0:512], sT[:, c0:512],
                                             AF.Exp)
                        if o >= 0:
                            nc.vector.tensor_tensor(pT[:, c0:c0 + 128],
                                                    pT[:, c0:c0 + 128],
                                                    tri_sb[:], op=MUL)
                        pts[j] = pT
                    for j in js:
                        pT = pts[j]
                        nc.tensor.matmul(dens[j][:, c0:512], ones_c,
                                         pT[:, c0:512],
                                         start=(kk == 0), stop=(kk == kmax - 1))
                        nc.tensor.matmul(ots[j][:, c0:512],
                                         vv[:, kk, j * 128:(j + 1) * 128],
                                         pT[:, c0:512],
                                         start=(kk == 0), stop=(kk == kmax - 1))
                for s, j in enumerate(js):
                    den, ot = dens[j], ots[j]
                    rden = t_o.tile([1, 512], F32R, name=f"rden_{qb}_{j}",
                                    tag=f"rden{s}")
                    with nc.allow_low_precision(reason="f32r = fp32 bits"):
                        nc.vector.reciprocal(rden[:], den[:])
                    bcp = t_bc.tile([128, 512], F32, name=f"bcp_{qb}_{j}",
                                    tag="bcp")
                    nc.tensor.matmul(bcp[:], ones_r[:1, :], rden[:],
                                     start=True, stop=True)
                    bcs = t_o.tile([128, 512], F32R, name=f"bcs_{qb}_{j}",
                                   tag=f"bcs{s}")
                    nc.vector.tensor_copy(bcs[:], bcp[:])
                    obf = t_o.tile([128, 512], BF16, name=f"obf_{qb}_{j}",
                                   tag=f"obf{s}")
                    nc.vector.tensor_tensor(obf[:], ots[j][:], bcs[:], op=MUL)
                    nc.sync.dma_start(
                        ag2_ins[qb][j * 128:(j + 1) * 128, :], obf[:])
            nc.gpsimd.collective_compute(
                "AllGather", mybir.AluOpType.bypass,
                ins=[ag2_ins[qb][:]], outs=[ag2_outs[qb][:]],
                replica_groups=[list(range(NCORES))],
            )
            if qb == 1:
                # prefetch phase_out's first input block while attention runs
                # (after qb1's collective: ag2[0] has long finished, so this
                # DMA runs immediately without stalling the queue behind it)
                oav0 = oa0[:].rearrange("p (k t) -> p k t", k=32)
                nc.sync.dma_start(
                    oav0, ag2_outs[0][:].rearrange("(k p) t -> p k t", p=128))


def _phase_out(nc, tc, io, ag2_outs, wov, oa0):
    """D-column-split output projection (bf16); wo preloaded upstream,
    tq=0 input prefetched during attention."""
    with (
        tc.tile_pool(name="o_a", bufs=2) as o_a,
        tc.tile_pool(name="o_st", bufs=3) as o_st,
        tc.tile_pool(name="o_ps", bufs=3, space="PSUM") as o_ps,
    ):
        for tq in range(4):
            if tq == 0:
                oav = oa0[:].rearrange("p (k t) -> p k t", k=32)
            else:
                oa = o_a.tile([128, 32 * 512], BF16, name=f"oa_{tq}", tag="oa")
                oav = oa[:].rearrange("p (k t) -> p k t", k=32)
                nc.sync.dma_start(
                    oav, ag2_outs[tq][:].rearrange("(k p) t -> p k t", p=128))
            for d in range(5):
                ps = o_ps.tile([128, 512], F32, name=f"ops_{tq}_{d}", tag="ops")
                for k in range(32):
                    nc.tensor.matmul(ps[:], wov[:, k, d * 128:(d + 1) * 128],
                                     oav[:, k, :], start=(k == 0), stop=(k == 31))
                st = o_st.tile([128, 512], F32, name=f"ost_{tq}_{d}", tag="ost")
                nc.vector.tensor_copy(st[:], ps[:])
                nc.sync.dma_start(
                    io["outT"][d * 128:(d + 1) * 128,
                               tq * 512:(tq + 1) * 512], st[:])


def _build():
    nc = bass.Bass("TRN2", target_bir_lowering=False, debug=False,
                   num_devices=NCORES)
    io = {
        "hT": nc.dram_tensor("hT", [D, TC], BF16, kind="ExternalInput"),
        "wa": nc.dram_tensor("wa", [D, QR + LAT], BF16, kind="ExternalInput"),
        "biasrep": nc.dram_tensor("biasrep", [128, LAT], F32,
                                  kind="ExternalInput"),
        "identb": nc.dram_tensor("identb", [128, 128], BF16,
                                 kind="ExternalInput"),
        "identf": nc.dram_tensor("identf", [128, 128], F32,
                                 kind="ExternalInput"),
        "wqb": nc.dram_tensor("wqb", [QR, 6 * 128], BF16, kind="ExternalInput"),
        "wkvbk": nc.dram_tensor("wkvbk", [KVR, HL * NOPE], BF16,
                                kind="ExternalInput"),
        "wkvbv": nc.dram_tensor("wkvbv", [KVR, HL * VH], BF16,
                                kind="ExternalInput"),
        "wo": nc.dram_tensor("wo", [H * VH, DCOL], BF16, kind="ExternalInput"),
        "cosT": nc.dram_tensor("cosT", [128, T], F32R, kind="ExternalInput"),
        "sinT": nc.dram_tensor("sinT", [128, T], F32R, kind="ExternalInput"),
        "cosAT": nc.dram_tensor("cosAT", [TC, 32], F32, kind="ExternalInput"),
        "sinAT": nc.dram_tensor("sinAT", [TC, 32], F32, kind="ExternalInput"),
        "tri": nc.dram_tensor("tri", [128, 128], F32R, kind="ExternalInput"),
        "onesin": nc.dram_tensor("onesin", [128, 128], F32R, kind="ExternalInput"),
        "outT": nc.dram_tensor("outT", [DCOL, T], F32, kind="ExternalOutput"),
    }

    with TileContext(nc) as tc:
        with (
            tc.tile_pool(name="dram", bufs=1, space="DRAM") as dram,
            tc.tile_pool(name="consts", bufs=1) as consts,
        ):
            agkv_in = dram.tile([LAT, TC], BF16, name="agkv_in")
            agkv_out = dram.tile([NCORES * LAT, TC], BF16, addr_space="Shared",
                                 name="agkv_out")
            agql_in = dram.tile([QR + 1, TC], BF16, name="agql_in")
            agql_out = dram.tile([NCORES * (QR + 1), TC], BF16, addr_space="Shared",
                                 name="agql_out")
            ag2_ins = [dram.tile([HL * VH, 512], BF16, name=f"ag2_in_{qb}")
                       for qb in range(4)]
            ag2_outs = [dram.tile([H * VH, 512], BF16, addr_space="Shared",
                                  name=f"ag2_out_{qb}") for qb in range(4)]

            consts_t = {}
            ones_sb = consts.tile([128, 128], F32R, name="ones_sb")
            nc.sync.dma_start(ones_sb[:], io["onesin"][:])
            consts_t["ones_c"] = ones_sb[:, 0:1]
            consts_t["ones_r"] = ones_sb
            ones_b = consts.tile([128, 1], BF16, name="ones_b")
            nc.vector.tensor_copy(ones_b[:], ones_sb[:, 0:1])
            consts_t["ones_cb"] = ones_b[:]
            trib = consts.tile([128, 128], BF16, name="trib")
            consts_t["tri_b"] = trib
            consts_t["tri_sb"] = consts.tile([128, 128], F32R, name="tri_sb")
            nc.sync.dma_start(consts_t["tri_sb"][:], io["tri"][:])
            nc.vector.tensor_copy(trib[:], consts_t["tri_sb"][:])
            onesrow_b = consts.tile([1, 128], BF16, name="onesrow_b")
            nc.vector.tensor_copy(onesrow_b[:], ones_sb[0:1, :])
            consts_t["onesrow_b"] = onesrow_b[:]

            agkvv = agkv_out[:].rearrange("(r a) t -> a r t", a=LAT)
            agqlv = agql_out[:].rearrange("(r a) t -> a r t", a=QR + 1)

            with tc.tile_pool(name="persist", bufs=1) as persist:
                kt_sb = persist.tile([128, HL * T], BF16, name="kt_sb")
                ktv = kt_sb[:].rearrange("p (j t) -> p j t", j=HL)
                v_sb = persist.tile([128, 16 * 512], BF16, name="v_sb")
                vv = v_sb[:].rearrange("p (mt c) -> p mt c", mt=16)
                kpe_sb = persist.tile([128, T], BF16, name="kpe_sb")
                qt_sb = persist.tile([128, 6 * T], BF16, name="qt_sb")

                with (
                    tc.tile_pool(name="b_kva", bufs=1) as b_kva,
                    tc.tile_pool(name="b_w", bufs=1) as b_w,
                ):
                    b_tiles = []

                    def prefetch_b():
                        b_tiles.extend(
                            _phase_b_loads(nc, io, agkvv, kpe_sb, b_kva, b_w))

                    with nc.named_scope("phase_a"):
                        _phase_a(nc, tc, io, consts_t, agkv_in, agkv_out,
                                 agql_in, agql_out)
                    with nc.named_scope("phase_q"):
                        _phase_q(nc, tc, io, consts_t, agqlv, qt_sb, prefetch_b)
                    with nc.named_scope("phase_b"):
                        _phase_b(nc, tc, ktv, vv, *b_tiles)

                with tc.tile_pool(name="opool", bufs=1) as opool:
                    wo_sb = opool.tile([128, 32 * DCOL], BF16, name="wo_sb")
                    wov = wo_sb[:].rearrange("p (k c) -> p k c", k=32)
                    nc.sync.dma_start(
                        wov, io["wo"][:].rearrange("(k p) c -> p k c", p=128))
                    oa0 = opool.tile([128, 32 * 512], BF16, name="oa0")

                    with nc.named_scope("phase_attn"):
                        _phase_attn(nc, tc, qt_sb, ag2_ins, ag2_outs,
                                    ktv, vv, kpe_sb, consts_t, oa0)

                    with nc.named_scope("phase_out"):
                        _phase_out(nc, tc, io, ag2_outs, wov, oa0)
    return nc


def _get_nc():
    if "nc" not in _cache:
        _cache["nc"] = _build()
    return _cache["nc"]


def _prep(inputs):
    h = np.asarray(inputs["h"], np.float32)
    pos = np.asarray(inputs["position_ids"], np.int32)
    Wq_a = np.asarray(inputs["Wq_a"], np.float32)
    gq = np.asarray(inputs["gq"], np.float32)
    Wq_b = np.asarray(inputs["Wq_b"], np.float32)
    Wkv_a = np.asarray(inputs["Wkv_a"], np.float32)
    bkv_a = np.asarray(inputs["bkv_a"], np.float32)
    gkv = np.asarray(inputs["gkv"], np.float32)
    Wkv_b = np.asarray(inputs["Wkv_b"], np.float32)
    Wo = np.asarray(inputs["Wo"], np.float32)

    dperm = np.concatenate([np.arange(0, ROPE, 2), np.arange(1, ROPE, 2)])
    scale = np.float32(1.0 / math.sqrt(QK))

    hT = np.ascontiguousarray(h.T)                      # [D, T]
    wkva = Wkv_a.copy()
    wkva[:, KVR:] = Wkv_a[:, KVR + dperm]
    bias = bkv_a.copy()
    bias[KVR:] = bkv_a[KVR + dperm]
    biasrep = np.ascontiguousarray(np.tile(bias[None, :], (128, 1)))

    wqb_eff = (Wq_b * gq[:, None]) * scale              # [QR, H*QK]
    wkvb_eff = Wkv_b * gkv[:, None]                     # [KVR, H*(NOPE+VH)]

    inv = THETA ** (-np.arange(0, ROPE, 2, dtype=np.float32) / ROPE)
    fr = pos.astype(np.float32)[:, None] * inv[None, :]  # [T, 32]
    cosF = np.cos(fr)
    sinF = np.sin(fr)
    cosT = np.ascontiguousarray(np.tile(cosF.T, (4, 1)))  # [128, T]
    sinT = np.ascontiguousarray(np.tile(sinF.T, (4, 1)))
    tri = np.triu(np.ones((128, 128), np.float32))
    bf16 = ml_dtypes.bfloat16
    wa = np.ascontiguousarray(
        np.concatenate([Wq_a, wkva], axis=1)).astype(bf16)  # [D, QR+LAT]
    identb = np.eye(128, dtype=bf16)
    identf = np.eye(128, dtype=np.float32)

    in_maps = []
    for c in range(NCORES):
        heads = list(range(HL * c, HL * (c + 1)))
        qcols = [np.arange(hh * QK, hh * QK + NOPE) for hh in heads]
        for pair in range(2):
            for hh in heads[2 * pair:2 * pair + 2]:
                qcols.append(hh * QK + NOPE + dperm)
        kcols = np.concatenate(
            [np.arange(hh * (NOPE + VH), hh * (NOPE + VH) + NOPE)
             for hh in heads])
        vcols = np.concatenate(
            [np.arange(hh * (NOPE + VH) + NOPE, (hh + 1) * (NOPE + VH))
             for hh in heads])
        in_maps.append({
            "hT": np.ascontiguousarray(hT[:, c * TC:(c + 1) * TC]).astype(bf16),
            "wa": wa,
            "biasrep": biasrep,
            "identb": identb,
            "identf": identf,
            "wqb": np.ascontiguousarray(wqb_eff[:, np.concatenate(qcols)]).astype(bf16),
            "wkvbk": np.ascontiguousarray(wkvb_eff[:, kcols]).astype(bf16),
            "wkvbv": np.ascontiguousarray(wkvb_eff[:, vcols]).astype(bf16),
            "wo": np.ascontiguousarray(Wo[:, c * DCOL:(c + 1) * DCOL]).astype(bf16),
            "cosT": cosT,
            "sinT": sinT,
            "cosAT": np.ascontiguousarray(cosF[c * TC:(c + 1) * TC, :]),
            "sinAT": np.ascontiguousarray(sinF[c * TC:(c + 1) * TC, :]),
            "tri": tri,
            "onesin": np.ones((128, 128), np.float32),
        })
    return in_maps


def kernel(**inputs):
    nc = _get_nc()
    in_maps = _prep(inputs)
    res = bass_utils.run_bass_kernel_spmd(
        nc, in_maps, core_ids=list(range(NCORES)), trace=TRACE[0])
    LAST_RESULT[0] = res
    out = np.empty((T, D), np.float32)
    for c in range(NCORES):
        out[:, c * DCOL:(c + 1) * DCOL] = res.results[c]["outT"].T
    return out


# revision 41
# speedup vs baseline: 1.0100x; 1.0100x over previous
"""DeepseekV2 MLA attention on 8 Trainium2 NeuronCores.

Sharding: token-split A-projections -> AllGather(kv latent, fired early) +
AllGather(q latent) -> head-split (4 heads/core) B-projections + causal
attention -> per-query-block AllGather(attn out) -> D-column-split output
projection. Layouts are d-major (feature dim on the SBUF partition axis).

Pipeline: the kv latent gather fires ~1/3 into phase_a so phase_b (k/v
projections) runs right after phase_a's matmuls; the q-latent gather is
hidden under phase_b; phase_q computes its rope chunks first so the
serialized vector rope chain overlaps the nope matmuls; q stays resident
in SBUF; the first o-projection input block is prefetched during
attention so phase_out starts without a DMA bubble.

Precision: bf16 matmul inputs with fp32 PSUM accumulation throughout;
rmsnorm statistics and softmax run in fp32/f32r.
"""
import math

import numpy as np
import ml_dtypes

import concourse.bass as bass
import concourse.mybir as mybir
from concourse.tile import TileContext
from concourse import bass_utils

# ---------------------------------------------------------------------------
# Walrus workaround: this container's walrus accepts at most ONE sync-wait
# per TPB instruction, but Tile attaches several (tail Drain, LDWEIGHTS...).
# Split: keep the last wait, move the rest onto preceding same-engine NOPs.
# ---------------------------------------------------------------------------
import concourse.tile as _tile_mod

_orig_sched = _tile_mod.TileContext.schedule_and_allocate
_nopctr = [0]


def _split_multiwait(nc):
    for fn in nc.m.functions:
        for blk in fn.blocks:
            insts = blk.instructions
            if not any(
                i.sync_info and i.sync_info.on_wait and len(i.sync_info.on_wait) > 1
                for i in insts
            ):
                continue
            out = []
            for ins in insts:
                si = ins.sync_info
                if si and si.on_wait and len(si.on_wait) > 1:
                    waits = list(si.on_wait)
                    for w in waits[:-1]:
                        _nopctr[0] += 1
                        nop = mybir.InstNoOp(name=f"I-mws-{_nopctr[0]}", ins=[], outs=[])
                        nop.engine = ins.engine
                        nop.sync_info = mybir.SyncInfo(on_wait=[w], on_update=[])
                        out.append(nop)
                    ins.sync_info = mybir.SyncInfo(
                        on_wait=[waits[-1]], on_update=list(si.on_update or [])
                    )
                out.append(ins)
            blk.instructions = out


def _patched_sched(self, *a, **k):
    res = _orig_sched(self, *a, **k)
    _split_multiwait(self.nc)
    return res


if getattr(_tile_mod.TileContext.schedule_and_allocate, "__name__", "") != "_patched_sched":
    _tile_mod.TileContext.schedule_and_allocate = _patched_sched


# ---------------------------------------------------------------------------
T, D, H = 2048, 5120, 32
NOPE, ROPE, QK = 128, 64, 192
KVR, QR, VH = 512, 1536, 128
EPS, THETA = 1e-6, 10000.0
NCORES = 8
HL = H // NCORES          # 4 heads per core
TC = T // NCORES          # 256 tokens per core
LAT = KVR + ROPE          # 576
DCOL = D // NCORES        # 640 output columns per core

F32 = mybir.dt.float32
F32R = mybir.dt.float32r
BF16 = mybir.dt.bfloat16
AF = mybir.ActivationFunctionType
MUL = mybir.AluOpType.mult
ADD = mybir.AluOpType.add
SUB = mybir.AluOpType.subtract

TRACE = [False]          # test.py sets TRACE[0]=True to profile
LAST_RESULT = [None]     # BassKernelResults stashed here for test.py

_cache = {}


def _phase_a(nc, tc, io, consts_t, agkv_in, agkv_out, agql_in, agql_out):
    """Token-split A projections with the h-chunk STATIONARY and the weight
    rows moving (512-wide matmuls; ~25% fewer PE cycles than the
    weight-stationary form, and no sum-of-squares / broadcast matmuls).
    Latents come out token-major; rmsnorm is a per-partition free-dim
    reduce; PE transposes restore d-major for the gathers.  The q latents
    ship UNNORMALIZED with the per-token 1/rms as an extra gathered row
    (applied consumer-side, folded into existing vector ops), so AG(q)
    fires right after the q matmuls with no normalization serialization."""
    with (
        tc.tile_pool(name="a_ht", bufs=1) as ht_pool,
        tc.tile_pool(name="a_cst", bufs=1) as a_cst,
        tc.tile_pool(name="a_w", bufs=3) as a_w,
        tc.tile_pool(name="a_st", bufs=2) as a_st,
        tc.tile_pool(name="a_tmp", bufs=2) as a_tmp,
        tc.tile_pool(name="a_ps", bufs=1, space="PSUM") as a_ps,
    ):
        ht_sb = ht_pool.tile([128, 40 * TC], BF16, name="ht_sb")
        htv = ht_sb[:].rearrange("p (k t) -> p k t", k=40)
        nc.sync.dma_start(htv, io["hT"][:].rearrange("(k p) t -> p k t", p=128))
        identb = a_cst.tile([128, 128], BF16, name="identb")
        nc.sync.dma_start(identb[:], io["identb"][:])
        identf = a_cst.tile([128, 128], F32, name="identf")
        nc.sync.dma_start(identf[:], io["identf"][:])
        biasr = a_cst.tile([128, LAT], F32, name="biasr")
        nc.sync.dma_start(biasr[:], io["biasrep"][:])
        cosat = a_cst.tile([128, 64], F32, name="cosat")
        sinat = a_cst.tile([128, 64], F32, name="sinat")
        for th in range(2):
            nc.sync.dma_start(cosat[:, th * 32:(th + 1) * 32],
                              io["cosAT"][th * 128:(th + 1) * 128, :])
            nc.sync.dma_start(sinat[:, th * 32:(th + 1) * 32],
                              io["sinAT"][th * 128:(th + 1) * 128, :])
        wav = io["wa"][:].rearrange("(k p) c -> p k c", p=128)

        def rms_scale(stin, nfeat, tag):
            """per-partition 1/sqrt(mean(stin^2)+eps) -> [128,1] f32r."""
            sq = a_st.tile([128, stin.shape[-1]], F32, name=f"sq_{tag}",
                           tag=f"sq_{tag}")
            nc.scalar.activation(sq[:], stin, AF.Square)
            ss = a_tmp.tile([128, 1], F32, name=f"ss_{tag}", tag=f"ss_{tag}")
            nc.vector.tensor_reduce(ss[:], sq[:], axis=mybir.AxisListType.X,
                                    op=ADD)
            ms = a_tmp.tile([128, 1], F32, name=f"ms_{tag}", tag=f"ms_{tag}")
            nc.vector.tensor_scalar(ms[:], ss[:], 1.0 / nfeat, EPS,
                                    op0=MUL, op1=ADD)
            sx = a_tmp.tile([128, 1], F32, name=f"sx_{tag}", tag=f"sx_{tag}")
            nc.scalar.activation(sx[:], ms[:], AF.Sqrt)
            rs = a_tmp.tile([128, 1], F32, name=f"rs_{tag}", tag=f"rs_{tag}")
            nc.vector.reciprocal(rs[:], sx[:])
            return rs

        def transpose_out(src_bf, cols, dst, dst_col0, th, tags):
            """PE-transpose [128, cols] bf16 (token-major) into d-major
            [cols, 128] and DMA to dst rows [dst_col0*...], token cols th."""
            nblk = (cols + 127) // 128
            for c in range(nblk):
                w = min(128, cols - c * 128)
                tp = a_ps.tile([128, 128], BF16, name=f"tp_{th}_{dst_col0}_{c}",
                               tag=tags[c % 2])
                nc.tensor.transpose(tp[:w, :], src_bf[:, c * 128:c * 128 + w],
                                    identb[:])
                tsb = a_tmp.tile([128, 128], BF16, name=f"tsb_{th}_{dst_col0}_{c}",
                                 tag="tsb")
                nc.vector.tensor_copy(tsb[:w, :], tp[:w, :])
                nc.sync.dma_start(
                    dst[dst_col0 + c * 128:dst_col0 + c * 128 + w,
                        th * 128:(th + 1) * 128], tsb[:w, :])

        # ---- pass 1: q columns (0..1535), both token halves ----
        psq = {}
        for th in range(2):
            for g in range(3):
                psq[th, g] = a_ps.tile([128, 512], F32, name=f"psq_{th}_{g}",
                                       tag=f"ps{th}{g}")
        for k in range(40):
            wt = a_w.tile([128, QR], BF16, name=f"aw_{k}", tag="aw")
            nc.sync.dma_start(wt[:, 0:768], wav[:, k, 0:768])
            nc.scalar.dma_start(wt[:, 768:QR], wav[:, k, 768:QR])
            for th in range(2):
                for g in range(3):
                    nc.tensor.matmul(psq[th, g][:],
                                     htv[:, k, th * 128:(th + 1) * 128],
                                     wt[:, g * 512:(g + 1) * 512],
                                     start=(k == 0), stop=(k == 39))
        qbfs = {}
        for th in range(2):
            qbf = a_st.tile([128, QR], BF16, name=f"qbf_{th}", tag="qbf")
            for g in range(3):
                nc.vector.tensor_copy(qbf[:, g * 512:(g + 1) * 512],
                                      psq[th, g][:])
            qbfs[th] = qbf
        srow = a_tmp.tile([1, TC], BF16, name="srow")
        for th in range(2):
            rs = rms_scale(qbfs[th][:], QR, f"q{th}")
            sps = a_ps.tile([1, 128], F32, name=f"sps_{th}", tag="pssc")
            nc.tensor.matmul(sps[:], rs[:], identf[:], start=True, stop=True)
            nc.vector.tensor_copy(srow[:, th * 128:(th + 1) * 128], sps[:])
        for th in range(2):
            transpose_out(qbfs[th][:], QR, agql_in, 0, th, ("ps00", "ps01"))
        nc.sync.dma_start(agql_in[QR:QR + 1, :], srow[:])
        with nc.named_scope("ag_ql"):
            nc.gpsimd.collective_compute(
                "AllGather", mybir.AluOpType.bypass,
                ins=[agql_in[:]], outs=[agql_out[:]],
                replica_groups=[list(range(NCORES))],
            )

        # ---- pass 2: kv columns (1536..2111) ----
        pskv = {}
        for th in range(2):
            pskv[th, 0] = a_ps.tile([128, 512], F32, name=f"pskv_{th}_0",
                                    tag=f"ps{th}0")
            pskv[th, 1] = a_ps.tile([128, 64], F32, name=f"pskv_{th}_1",
                                    tag=f"ps{th}1")
        for k in range(40):
            wt2 = a_w.tile([128, LAT], BF16, name=f"awkv_{k}", tag="awkv")
            nc.sync.dma_start(wt2[:, 0:288], wav[:, k, QR:QR + 288])
            nc.scalar.dma_start(wt2[:, 288:LAT], wav[:, k, QR + 288:QR + LAT])
            for th in range(2):
                nc.tensor.matmul(pskv[th, 0][:],
                                 htv[:, k, th * 128:(th + 1) * 128],
                                 wt2[:, 0:512], start=(k == 0), stop=(k == 39))
                nc.tensor.matmul(pskv[th, 1][:],
                                 htv[:, k, th * 128:(th + 1) * 128],
                                 wt2[:, 512:576], start=(k == 0), stop=(k == 39))
        for th in range(2):
            stkv = a_st.tile([128, LAT], F32, name=f"stkv_{th}", tag="stkv")
            nc.vector.tensor_tensor(stkv[:, 0:512], pskv[th, 0][:],
                                    biasr[:, 0:512], op=ADD)
            nc.vector.tensor_tensor(stkv[:, 512:576], pskv[th, 1][:],
                                    biasr[:, 512:576], op=ADD)
            rskv = rms_scale(stkv[:, 0:512], KVR, f"kv{th}")
            kvbf = a_tmp.tile([128, LAT], BF16, name=f"kvbf_{th}", tag="kvbf")
            nc.vector.tensor_scalar(kvbf[:, 0:512], stkv[:, 0:512], rskv[:],
                                    None, op0=MUL)
            x1, x2 = stkv[:, 512:544], stkv[:, 544:576]
            ca = cosat[:, th * 32:(th + 1) * 32]
            sa = sinat[:, th * 32:(th + 1) * 32]
            ct1 = a_tmp.tile([128, 32], F32, name=f"ct1_{th}", tag="ct1")
            ct2 = a_tmp.tile([128, 32], F32, name=f"ct2_{th}", tag="ct2")
            nc.vector.tensor_tensor(ct1[:], x1, ca, op=MUL)
            nc.vector.tensor_tensor(ct2[:], x2, sa, op=MUL)
            nc.vector.tensor_tensor(kvbf[:, 512:544], ct1[:], ct2[:], op=SUB)
            nc.vector.tensor_tensor(ct1[:], x1, sa, op=MUL)
            nc.vector.tensor_tensor(ct2[:], x2, ca, op=MUL)
            nc.vector.tensor_tensor(kvbf[:, 544:576], ct1[:], ct2[:], op=ADD)
            transpose_out(kvbf[:, 0:512], 512, agkv_in, 0, th, ("ps02", "ps12"))
            transpose_out(kvbf[:, 512:576], 64, agkv_in, 512, th,
                          ("ps02", "ps12"))
        with nc.named_scope("ag_kv"):
            nc.gpsimd.collective_compute(
                "AllGather", mybir.AluOpType.bypass,
                ins=[agkv_in[:]], outs=[agkv_out[:]],
                replica_groups=[list(range(NCORES))],
            )


def _phase_b_loads(nc, io, agkvv, kpe_sb, b_kva, b_w):
    """DMA the gathered kv latents + B-projection weights; returns tiles.
    Issued on the SCALAR hw-dma queue so they don't head-of-line block
    the sync queue (they wait on AG(kv), which lands late but with
    slack — phase_b's matmuls run after phase_q's)."""
    wk_sb = b_w.tile([128, 4 * 512], BF16, name="wk_sb", tag="wkw")
    nc.scalar.dma_start(wk_sb[:].rearrange("p (k c) -> p k c", k=4),
                        io["wkvbk"][:].rearrange("(k p) c -> p k c", p=128))
    kva_sb = b_kva.tile([128, 4 * T], BF16, name="kva_sb")
    kvav = kva_sb[:].rearrange("p (k t) -> p k t", k=4)
    for k in range(4):
        nc.scalar.dma_start(
            kvav[:, k, :].rearrange("p (r t) -> p r t", r=NCORES),
            agkvv[k * 128:(k + 1) * 128])
    # k_pe duplicated on both partition halves so attention can feed
    # matmuls whose q slice lives at base partition 0 or 64
    for half in range(2):
        nc.scalar.dma_start(
            kpe_sb[64 * half:64 * half + 64, :].rearrange(
                "p (r t) -> p r t", r=NCORES),
            agkvv[512:576])
    wv_sb = b_w.tile([128, 4 * 512], BF16, name="wv_sb", tag="wvw")
    nc.scalar.dma_start(wv_sb[:].rearrange("p (k c) -> p k c", k=4),
                        io["wkvbv"][:].rearrange("(k p) c -> p k c", p=128))
    return kva_sb, wk_sb, wv_sb


def _phase_b(nc, tc, ktv, vv, kva_sb, wk_sb, wv_sb):
    """Head-split k_nope^T and v projections from the gathered kv latents."""
    kvav = kva_sb[:].rearrange("p (k t) -> p k t", k=4)
    wkv_ = wk_sb[:].rearrange("p (k c) -> p k c", k=4)
    wvv = wv_sb[:].rearrange("p (k c) -> p k c", k=4)
    with tc.tile_pool(name="b_ps", bufs=2, space="PSUM") as b_ps:
        for j in range(HL):
            for qb in range(4):
                ps = b_ps.tile([128, 512], F32, name=f"psk_{j}_{qb}", tag="psk")
                for k in range(4):
                    nc.tensor.matmul(ps[:], wkv_[:, k, j * 128:(j + 1) * 128],
                                     kvav[:, k, qb * 512:(qb + 1) * 512],
                                     start=(k == 0), stop=(k == 3))
                nc.vector.tensor_copy(ktv[:, j, qb * 512:(qb + 1) * 512], ps[:])
        for mt in range(16):
            ps = b_ps.tile([128, 512], F32, name=f"psv_{mt}", tag="psv")
            for k in range(4):
                nc.tensor.matmul(ps[:], kvav[:, k, mt * 128:(mt + 1) * 128],
                                 wvv[:, k, :], start=(k == 0), stop=(k == 3))
            nc.vector.tensor_copy(vv[:, mt, :], ps[:])


def _phase_q(nc, tc, io, consts_t, agqlv, qt_sb, prefetch_cb):
    """Head-split q^T projection into resident SBUF qt; rope (pe) chunks
    are computed FIRST so the serialized vector rope chain overlaps the
    nope matmuls that follow."""
    qtv = qt_sb[:].rearrange("p (c t) -> p c t", c=6)
    with (
        tc.tile_pool(name="c_qa", bufs=1) as c_qa,
        tc.tile_pool(name="c_tab", bufs=1) as c_tab,
        tc.tile_pool(name="c_tmp", bufs=1) as c_tmp,
        tc.tile_pool(name="c_ps", bufs=2, space="PSUM") as c_ps,
    ):
        # gathered q latents: 12 chunks split across the two HW DMA queues
        # (Sync + Activation) so the strided gathers land ~2x faster
        qa_sb = c_qa.tile([128, 12 * T], BF16, name="qa_sb")
        qav = qa_sb[:].rearrange("p (k t) -> p k t", k=12)
        for k in range(12):
            eng = nc.sync if k % 2 == 0 else nc.scalar
            eng.dma_start(
                qav[:, k, :].rearrange("p (r t) -> p r t", r=NCORES),
                agqlv[k * 128:(k + 1) * 128])
        prefetch_cb()
        sbc = c_tab.tile([128, T], F32R, name="sbc")
        cos2 = c_tab.tile([128, T], F32R, name="cos2")
        sin2 = c_tab.tile([128, T], F32R, name="sin2")
        ones_r = consts_t["ones_r"]
        with tc.tile_pool(name="c_cs", bufs=1) as c_cs:
            # per-token 1/rms scale row (row QR of every rank block) ->
            # broadcast to 128 partitions, folded into cos/sin tables and
            # the nope copy-multiplies below
            srow_sb = c_cs.tile([1, T], BF16, name="srow_sb")
            nc.sync.dma_start(
                srow_sb[:].rearrange("p (r t) -> p r t", r=NCORES),
                agqlv[QR:QR + 1])
            srow_f = c_cs.tile([1, T], F32R, name="srow_f")
            nc.vector.tensor_copy(srow_f[:], srow_sb[:])
            for qq in range(4):
                bps = c_ps.tile([128, 512], F32, name=f"sbps_{qq}",
                                tag=f"psq{qq}")
                nc.tensor.matmul(bps[:], ones_r[:1, :],
                                 srow_f[:, qq * 512:(qq + 1) * 512],
                                 start=True, stop=True)
                nc.vector.tensor_copy(sbc[:, qq * 512:(qq + 1) * 512], bps[:])
            cos_sb = c_cs.tile([128, T], F32R, name="cos_sb")
            sin_sb = c_cs.tile([128, T], F32R, name="sin_sb")
            nc.sync.dma_start(cos_sb[:], io["cosT"][:])
            nc.sync.dma_start(sin_sb[:], io["sinT"][:])
            nc.vector.tensor_tensor(cos2[:], cos_sb[:], sbc[:], op=MUL)
            nc.vector.tensor_tensor(sin2[:], sin_sb[:], sbc[:], op=MUL)
        ctx_w = tc.tile_pool(name="c_w", bufs=6)
        c_w = ctx_w.__enter__()
        morder = (4, 5, 0, 1, 2, 3)   # pe chunks first
        wts = {}
        for m in morder:
            wt = c_w.tile([128, 12 * 128], BF16, name=f"cw_{m}", tag="cw")
            nc.sync.dma_start(
                wt[:].rearrange("p (k c) -> p k c", k=12),
                io["wqb"][:].rearrange("(k p) c -> p k c", p=128)[
                    :, :, m * 128:(m + 1) * 128])
            wts[m] = wt
        pestage = c_tab.tile([128, 2 * T], F32R, name="pestage")

        def rope_batch(m, qb):
            st = qtv[:, m, qb * 512:(qb + 1) * 512]
            pe = pestage[:, (m - 4) * T + qb * 512:(m - 4) * T + (qb + 1) * 512]
            cs = cos2[:, qb * 512:(qb + 1) * 512]
            sn = sin2[:, qb * 512:(qb + 1) * 512]
            for half in range(2):
                r0 = 64 * half
                x1 = pe[r0:r0 + 32, :]
                x2 = pe[r0 + 32:r0 + 64, :]
                t1 = c_tmp.tile([32, 512], F32R,
                                name=f"ct1_{m}_{qb}_{half}", tag="ct1")
                t2 = c_tmp.tile([32, 512], F32R,
                                name=f"ct2_{m}_{qb}_{half}", tag="ct2")
                nc.vector.tensor_tensor(t1[:], x1, cs[r0:r0 + 32, :], op=MUL)
                nc.vector.tensor_tensor(t2[:], x2, sn[r0 + 32:r0 + 64, :],
                                        op=MUL)
                nc.vector.tensor_tensor(st[r0:r0 + 32, :], t1[:], t2[:],
                                        op=SUB)
                nc.vector.tensor_tensor(t1[:], x1, sn[r0:r0 + 32, :], op=MUL)
                nc.vector.tensor_tensor(t2[:], x2, cs[r0 + 32:r0 + 64, :],
                                        op=MUL)
                nc.vector.tensor_tensor(st[r0 + 32:r0 + 64, :], t1[:], t2[:],
                                        op=ADD)

        # rope batches are interleaved between the nope chunks' PSUM copies
        # on the vector FIFO: each batch (~16us) paces one nope chunk's
        # matmuls (~15us) without ever blocking a PSUM hand-off
        rope_work = [(m, qb) for qb in range(4) for m in (4, 5)]
        for m in morder:
            wtv = wts[m][:].rearrange("p (k c) -> p k c", k=12)
            pss = [c_ps.tile([128, 512], F32, name=f"psq_{m}_{qb}", tag=f"psq{qb}")
                   for qb in range(4)]
            for k in range(12):
                for qb in range(4):
                    nc.tensor.matmul(pss[qb][:], wtv[:, k, :],
                                     qav[:, k, qb * 512:(qb + 1) * 512],
                                     start=(k == 0), stop=(k == 11))
            for qb in range(4):
                if m < 4:
                    nc.vector.tensor_tensor(qtv[:, m, qb * 512:(qb + 1) * 512],
                                            pss[qb][:],
                                            sbc[:, qb * 512:(qb + 1) * 512],
                                            op=MUL)
                else:
                    pe = pestage[:, (m - 4) * T + qb * 512:
                                 (m - 4) * T + (qb + 1) * 512]
                    nc.vector.tensor_copy(pe, pss[qb][:])
            if m < 4 and rope_work:
                rope_batch(*rope_work.pop(0))
                rope_batch(*rope_work.pop(0))
        for mq in rope_work:
            rope_batch(*mq)
        ctx_w.__exit__(None, None, None)


def _phase_attn(nc, tc, qt_sb, ag2_ins, ag2_outs, ktv, vv, kpe_sb, consts_t,
                oa0):
    """Causal attention, two heads interleaved per pass; bf16 out -> ag2_in.
    q is read directly from resident SBUF (qt_sb)."""
    ones_c, ones_r, tri_sb = (consts_t["ones_cb"], consts_t["ones_r"],
                              consts_t["tri_b"])
    with (
        tc.tile_pool(name="t_p", bufs=8) as t_p,
        tc.tile_pool(name="t_o", bufs=2) as t_o,
        tc.tile_pool(name="t_ps", bufs=3, space="PSUM") as t_ps,
        tc.tile_pool(name="t_bc", bufs=1, space="PSUM") as t_bc,
        tc.tile_pool(name="t_acc", bufs=1, space="PSUM") as t_acc,
    ):
        for qb in range(4):
            for jp in range(HL // 2):
                js = (2 * jp, 2 * jp + 1)
                qf = {}
                dens, ots = {}, {}
                for s, j in enumerate(js):
                    qfn = qt_sb[:, j * T + qb * 512:j * T + (qb + 1) * 512]
                    pc = (4 + j // 2) * T + qb * 512
                    r0 = 64 * (j % 2)
                    qfp = qt_sb[r0:r0 + 64, pc:pc + 512]
                    qf[j] = (qfn, qfp)
                    dens[j] = t_acc.tile([1, 512], F32, name=f"den_{qb}_{j}",
                                         tag=f"den{s}")
                    ots[j] = t_acc.tile([128, 512], F32, name=f"ot_{qb}_{j}",
                                        tag=f"ot{s}")
                kmax = 4 * qb + 4
                for kk in range(kmax):
                    o = kk - 4 * qb
                    c0 = max(0, o) * 128
                    pts = {}
                    for s, j in enumerate(js):
                        qfn, qfp = qf[j]
                        sT = t_ps.tile([128, 512], F32,
                                       name=f"sT_{qb}_{j}_{kk}", tag="sT")
                        nc.tensor.matmul(sT[:, c0:512],
                                         ktv[:, j, kk * 128:(kk + 1) * 128],
                                         qfn[:, c0:512], start=True, stop=False)
                        r0 = 64 * (j % 2)
                        nc.tensor.matmul(sT[:, c0:512],
                                         kpe_sb[r0:r0 + 64,
                                                kk * 128:(kk + 1) * 128],
                                         qfp[:, c0:512], start=False, stop=True)
                        pT = t_p.tile([128, 512], BF16,
                                      name=f"pT_{qb}_{j}_{kk}", tag="pT")
                        nc.scalar.activation(pT[:, c0:512], sT[:, c0:512],
                                             AF.Exp)
                        if o >= 0:
                            nc.vector.tensor_tensor(pT[:, c0:c0 + 128],
                                                    pT[:, c0:c0 + 128],
                                                    tri_sb[:], op=MUL)
                        pts[j] = pT
                    for j in js:
                        pT = pts[j]
                        nc.tensor.matmul(dens[j][:, c0:512], ones_c,
                                         pT[:, c0:512],
                                         start=(kk == 0), stop=(kk == kmax - 1))
                        nc.tensor.matmul(ots[j][:, c0:512],
                                         vv[:, kk, j * 128:(j + 1) * 128],
                                         pT[:, c0:512],
                                         start=(kk == 0), stop=(kk == kmax - 1))
                for s, j in enumerate(js):
                    den, ot = dens[j], ots[j]
                    rden = t_o.tile([1, 512], F32R, name=f"rden_{qb}_{j}",
                                    tag=f"rden{s}")
                    with nc.allow_low_precision(reason="f32r = fp32 bits"):
                        nc.vector.reciprocal(rden[:], den[:])
                    bcp = t_bc.tile([128, 512], F32, name=f"bcp_{qb}_{j}",
                                    tag="bcp")
                    nc.tensor.matmul(bcp[:], ones_r[:1, :], rden[:],
                                     start=True, stop=True)
                    bcs = t_o.tile([128, 512], F32R, name=f"bcs_{qb}_{j}",
                                   tag=f"bcs{s}")
                    nc.vector.tensor_copy(bcs[:], bcp[:])
                    obf = t_o.tile([128, 512], BF16, name=f"obf_{qb}_{j}",
                                   tag=f"obf{s}")
                    nc.vector.tensor_tensor(obf[:], ots[j][:], bcs[:], op=MUL)
                    nc.sync.dma_start(
                        ag2_ins[qb][j * 128:(j + 1) * 128, :], obf[:])
            nc.gpsimd.collective_compute(
                "AllGather", mybir.AluOpType.bypass,
                ins=[ag2_ins[qb][:]], outs=[ag2_outs[qb][:]],
                replica_groups=[list(range(NCORES))],
            )
            if qb == 1:
                # prefetch phase_out's first input block while attention runs
                # (after qb1's collective: ag2[0] has long finished, so this
                # DMA runs immediately without stalling the queue behind it)
                oav0 = oa0[:].rearrange("p (k t) -> p k t", k=32)
                nc.sync.dma_start(
                    oav0, ag2_outs[0][:].rearrange("(k p) t -> p k t", p=128))


def _phase_out(nc, tc, io, ag2_outs, wov, oa0):
    """D-column-split output projection (bf16); wo preloaded upstream,
    tq=0 input prefetched during attention."""
    with (
        tc.tile_pool(name="o_a", bufs=2) as o_a,
        tc.tile_pool(name="o_st", bufs=3) as o_st,
        tc.tile_pool(name="o_ps", bufs=3, space="PSUM") as o_ps,
    ):
        for tq in range(4):
            if tq == 0:
                oav = oa0[:].rearrange("p (k t) -> p k t", k=32)
            else:
                oa = o_a.tile([128, 32 * 512], BF16, name=f"oa_{tq}", tag="oa")
                oav = oa[:].rearrange("p (k t) -> p k t", k=32)
                nc.sync.dma_start(
                    oav, ag2_outs[tq][:].rearrange("(k p) t -> p k t", p=128))
            for d in range(5):
                ps = o_ps.tile([128, 512], F32, name=f"ops_{tq}_{d}", tag="ops")
                for k in range(32):
                    nc.tensor.matmul(ps[:], wov[:, k, d * 128:(d + 1) * 128],
                                     oav[:, k, :], start=(k == 0), stop=(k == 31))
                st = o_st.tile([128, 512], F32, name=f"ost_{tq}_{d}", tag="ost")
                nc.vector.tensor_copy(st[:], ps[:])
                nc.sync.dma_start(
                    io["outT"][d * 128:(d + 1) * 128,
                               tq * 512:(tq + 1) * 512], st[:])


def _build():
    nc = bass.Bass("TRN2", target_bir_lowering=False, debug=False,
                   num_devices=NCORES)
    io = {
        "hT": nc.dram_tensor("hT", [D, TC], BF16, kind="ExternalInput"),
        "wa": nc.dram_tensor("wa", [D, QR + LAT], BF16, kind="ExternalInput"),
        "biasrep": nc.dram_tensor("biasrep", [128, LAT], F32,
                                  kind="ExternalInput"),
        "identb": nc.dram_tensor("identb", [128, 128], BF16,
                                 kind="ExternalInput"),
        "identf": nc.dram_tensor("identf", [128, 128], F32,
                                 kind="ExternalInput"),
        "wqb": nc.dram_tensor("wqb", [QR, 6 * 128], BF16, kind="ExternalInput"),
        "wkvbk": nc.dram_tensor("wkvbk", [KVR, HL * NOPE], BF16,
                                kind="ExternalInput"),
        "wkvbv": nc.dram_tensor("wkvbv", [KVR, HL * VH], BF16,
                                kind="ExternalInput"),
        "wo": nc.dram_tensor("wo", [H * VH, DCOL], BF16, kind="ExternalInput"),
        "cosT": nc.dram_tensor("cosT", [128, T], F32R, kind="ExternalInput"),
        "sinT": nc.dram_tensor("sinT", [128, T], F32R, kind="ExternalInput"),
        "cosAT": nc.dram_tensor("cosAT", [TC, 32], F32, kind="ExternalInput"),
        "sinAT": nc.dram_tensor("sinAT", [TC, 32], F32, kind="ExternalInput"),
        "tri": nc.dram_tensor("tri", [128, 128], F32R, kind="ExternalInput"),
        "onesin": nc.dram_tensor("onesin", [128, 128], F32R, kind="ExternalInput"),
        "outT": nc.dram_tensor("outT", [DCOL, T], F32, kind="ExternalOutput"),
    }

    with TileContext(nc) as tc:
        with (
            tc.tile_pool(name="dram", bufs=1, space="DRAM") as dram,
            tc.tile_pool(name="consts", bufs=1) as consts,
        ):
            agkv_in = dram.tile([LAT, TC], BF16, name="agkv_in")
            agkv_out = dram.tile([NCORES * LAT, TC], BF16, addr_space="Shared",
                                 name="agkv_out")
            agql_in = dram.tile([QR + 1, TC], BF16, name="agql_in")
            agql_out = dram.tile([NCORES * (QR + 1), TC], BF16, addr_space="Shared",
                                 name="agql_out")
            ag2_ins = [dram.tile([HL * VH, 512], BF16, name=f"ag2_in_{qb}")
                       for qb in range(4)]
            ag2_outs = [dram.tile([H * VH, 512], BF16, addr_space="Shared",
                                  name=f"ag2_out_{qb}") for qb in range(4)]

            consts_t = {}
            ones_sb = consts.tile([128, 128], F32R, name="ones_sb")
            nc.sync.dma_start(ones_sb[:], io["onesin"][:])
            consts_t["ones_c"] = ones_sb[:, 0:1]
            consts_t["ones_r"] = ones_sb
            ones_b = consts.tile([128, 1], BF16, name="ones_b")
            nc.vector.tensor_copy(ones_b[:], ones_sb[:, 0:1])
            consts_t["ones_cb"] = ones_b[:]
            trib = consts.tile([128, 128], BF16, name="trib")
            consts_t["tri_b"] = trib
            consts_t["tri_sb"] = consts.tile([128, 128], F32R, name="tri_sb")
            nc.sync.dma_start(consts_t["tri_sb"][:], io["tri"][:])
            nc.vector.tensor_copy(trib[:], consts_t["tri_sb"][:])
            onesrow_b = consts.tile([1, 128], BF16, name="onesrow_b")
            nc.vector.tensor_copy(onesrow_b[:], ones_sb[0:1, :])
            consts_t["onesrow_b"] = onesrow_b[:]

            agkvv = agkv_out[:].rearrange("(r a) t -> a r t", a=LAT)
            agqlv = agql_out[:].rearrange("(r a) t -> a r t", a=QR + 1)

            with tc.tile_pool(name="persist", bufs=1) as persist:
                kt_sb = persist.tile([128, HL * T], BF16, name="kt_sb")
                ktv = kt_sb[:].rearrange("p (j t) -> p j t", j=HL)
                v_sb = persist.tile([128, 16 * 512], BF16, name="v_sb")
                vv = v_sb[:].rearrange("p (mt c) -> p mt c", mt=16)
                kpe_sb = persist.tile([128, T], BF16, name="kpe_sb")
                qt_sb = persist.tile([128, 6 * T], BF16, name="qt_sb")

                with (
                    tc.tile_pool(name="b_kva", bufs=1) as b_kva,
                    tc.tile_pool(name="b_w", bufs=1) as b_w,
                ):
                    b_tiles = []

                    def prefetch_b():
                        b_tiles.extend(
                            _phase_b_loads(nc, io, agkvv, kpe_sb, b_kva, b_w))

                    with nc.named_scope("phase_a"):
                        _phase_a(nc, tc, io, consts_t, agkv_in, agkv_out,
                                 agql_in, agql_out)
                    with nc.named_scope("phase_q"):
                        _phase_q(nc, tc, io, consts_t, agqlv, qt_sb, prefetch_b)
                    with nc.named_scope("phase_b"):
                        _phase_b(nc, tc, ktv, vv, *b_tiles)

                with tc.tile_pool(name="opool", bufs=1) as opool:
                    wo_sb = opool.tile([128, 32 * DCOL], BF16, name="wo_sb")
                    wov = wo_sb[:].rearrange("p (k c) -> p k c", k=32)
                    nc.sync.dma_start(
                        wov, io["wo"][:].rearrange("(k p) c -> p k c", p=128))
                    oa0 = opool.tile([128, 32 * 512], BF16, name="oa0")

                    with nc.named_scope("phase_attn"):
                        _phase_attn(nc, tc, qt_sb, ag2_ins, ag2_outs,
                                    ktv, vv, kpe_sb, consts_t, oa0)

                    with nc.named_scope("phase_out"):
                        _phase_out(nc, tc, io, ag2_outs, wov, oa0)
    return nc


def _get_nc():
    if "nc" not in _cache:
        _cache["nc"] = _build()
    return _cache["nc"]


def _prep(inputs):
    h = np.asarray(inputs["h"], np.float32)
    pos = np.asarray(inputs["position_ids"], np.int32)
    Wq_a = np.asarray(inputs["Wq_a"], np.float32)
    gq = np.asarray(inputs["gq"], np.float32)
    Wq_b = np.asarray(inputs["Wq_b"], np.float32)
    Wkv_a = np.asarray(inputs["Wkv_a"], np.float32)
    bkv_a = np.asarray(inputs["bkv_a"], np.float32)
    gkv = np.asarray(inputs["gkv"], np.float32)
    Wkv_b = np.asarray(inputs["Wkv_b"], np.float32)
    Wo = np.asarray(inputs["Wo"], np.float32)

    dperm = np.concatenate([np.arange(0, ROPE, 2), np.arange(1, ROPE, 2)])
    scale = np.float32(1.0 / math.sqrt(QK))

    hT = np.ascontiguousarray(h.T)                      # [D, T]
    wkva = Wkv_a.copy()
    wkva[:, KVR:] = Wkv_a[:, KVR + dperm]
    bias = bkv_a.copy()
    bias[KVR:] = bkv_a[KVR + dperm]
    biasrep = np.ascontiguousarray(np.tile(bias[None, :], (128, 1)))

    wqb_eff = (Wq_b * gq[:, None]) * scale              # [QR, H*QK]
    wkvb_eff = Wkv_b * gkv[:, None]                     # [KVR, H*(NOPE+VH)]

    inv = THETA ** (-np.arange(0, ROPE, 2, dtype=np.float32) / ROPE)
    fr = pos.astype(np.float32)[:, None] * inv[None, :]  # [T, 32]
    cosF = np.cos(fr)
    sinF = np.sin(fr)
    cosT = np.ascontiguousarray(np.tile(cosF.T, (4, 1)))  # [128, T]
    sinT = np.ascontiguousarray(np.tile(sinF.T, (4, 1)))
    tri = np.triu(np.ones((128, 128), np.float32))
    bf16 = ml_dtypes.bfloat16
    wa = np.ascontiguousarray(
        np.concatenate([Wq_a, wkva], axis=1)).astype(bf16)  # [D, QR+LAT]
    identb = np.eye(128, dtype=bf16)
    identf = np.eye(128, dtype=np.float32)

    in_maps = []
    for c in range(NCORES):
        heads = list(range(HL * c, HL * (c + 1)))
        qcols = [np.arange(hh * QK, hh * QK + NOPE) for hh in heads]
        for pair in range(2):
            for hh in heads[2 * pair:2 * pair + 2]:
                qcols.append(hh * QK + NOPE + dperm)
        kcols = np.concatenate(
            [np.arange(hh * (NOPE + VH), hh * (NOPE + VH) + NOPE)
             for hh in heads])
        vcols = np.concatenate(
            [np.arange(hh * (NOPE + VH) + NOPE, (hh + 1) * (NOPE + VH))
             for hh in heads])
        in_maps.append({
            "hT": np.ascontiguousarray(hT[:, c * TC:(c + 1) * TC]).astype(bf16),
            "wa": wa,
            "biasrep": biasrep,
            "identb": identb,
            "identf": identf,
            "wqb": np.ascontiguousarray(wqb_eff[:, np.concatenate(qcols)]).astype(bf16),
            "wkvbk": np.ascontiguousarray(wkvb_eff[:, kcols]).astype(bf16),
            "wkvbv": np.ascontiguousarray(wkvb_eff[:, vcols]).astype(bf16),
            "wo": np.ascontiguousarray(Wo[:, c * DCOL:(c + 1) * DCOL]).astype(bf16),
            "cosT": cosT,
            "sinT": sinT,
            "cosAT": np.ascontiguousarray(cosF[c * TC:(c + 1) * TC, :]),
            "sinAT": np.ascontiguousarray(sinF[c * TC:(c + 1) * TC, :]),
            "tri": tri,
            "onesin": np.ones((128, 128), np.float32),
        })
    return in_maps


def kernel(**inputs):
    nc = _get_nc()
    in_maps = _prep(inputs)
    res = bass_utils.run_bass_kernel_spmd(
        nc, in_maps, core_ids=list(range(NCORES)), trace=TRACE[0])
    LAST_RESULT[0] = res
    out = np.empty((T, D), np.float32)
    for c in range(NCORES):
        out[:, c * DCOL:(c + 1) * DCOL] = res.results[c]["outT"].T
    return out
